# revision 9
# baseline (speedup 1.0000x reference)
"""Multi-head self-attention 2d kernel for 8 trn2 NeuronCores.

Sharding: data-parallel over batch B=16 -> 2 batches per core.

v2: all four projections run as fp8e4m3 DoubleRowSwInterleave matmuls
(host pre-interleaves the stationary operand, so LDWEIGHTS reads
contiguously and each 512-col matmul covers two 128-deep k-tiles).  exp
tiles are fp8e4m3 (3 mantissa bits vs e5m2's 2), which pays for the fp8
projection noise.  bv is folded into bo on the host (bo_eff = bo+wo@bv),
so the v write is a scaled psum->sbuf copy on the scalar engine.  The
out-projection bias enters PSUM via a K=1 ones matmul and the
scale+residual is one scalar_tensor_tensor on the vector engine.

Per-core dataflow (per batch):
  q = (S_W*(wq8@x8) + S_W*bq)*(A_Q/S_W) -> [C, N] bf16   (A_Q folds the
      e4m3 exp2-bit-trick scale into q)
  k analogously (scale 1/S_W)
  vT tiles [N, 2, 8, 66] fp8e4m3 (64 d + ones row + pad per head),
      written by scalar-engine Copy with scale S_V/S_W
  per head pair hp (software-pipelined one pair behind):
    eT[j, i] = k_h.T @ q_h            (bf16, K=64, row-tiled head pair)
    expT = exp(SCALE*eT - C_BIAS) fp8e4m3  (scalar ACT; some tiles via a
           uint8 exp2 bit trick on the vector engine)
    pu[0:65, i] = v_h.T @ expT        (fp8 DoubleRow over jt pairs;
                                       row 64 = denominator)
    r = 1/denom broadcast via K=1 bf16 matmul + fast reciprocal
    on8 = pu[0:64] * r -> fp8e4m3 (= S_V * out_norm)
  y = (wo8@on8 + 512*bo_eff)*gamma/512 + x   (fp8 DR matmuls, K=1 bias
      matmul, one scalar_tensor_tensor for scale+residual)
"""

import sys

for _p in ("/opt/trn_rl_repo",):
    if _p not in sys.path:
        sys.path.insert(0, _p)

import numpy as np
import ml_dtypes

import concourse.bass as bass
from concourse import bacc
import concourse.mybir as mybir
import concourse.tile as tile
from concourse.bass_utils import run_bass_kernel_spmd

F32 = mybir.dt.float32
BF16 = mybir.dt.bfloat16
F8E4 = mybir.dt.float8e4
U8 = mybir.dt.uint8
AF = mybir.ActivationFunctionType
ALU = mybir.AluOpType
DR = mybir.MatmulPerfMode.DoubleRow
DRSWI = mybir.MatmulPerfMode.DoubleRowSwInterleave

C = 512
N = 1024
HEADS = 8
HD = C // HEADS  # 64
SCALE = HD ** -0.5
CT = C // 128  # 4 channel tiles
CTP = CT // 2  # 2 channel tile pairs (fp8 DR k-tile pairs)
NT = N // 128  # 8 spatial tiles
NTP = NT // 2  # 4 spatial tile pairs
NCH = N // 512  # 2 free-dim chunks
BPC = 2  # batches per core
NCORES = 8
VW = HD + 2  # v tile row stride per head: 64 d + ones + pad
S_W = 16.0  # weight quantization scale (w8 = S_W * w in e4m3)
S_V = 32.0  # v quantization scale (v8 = S_V * v); on8 = S_V * out_norm
C_BIAS = 3.0  # exp bias, cancels in the normalization (keeps exp < 240, e4m3-safe)
LOG2E = float(np.log2(np.e))
A_Q = SCALE * LOG2E * 8.0  # e4m3 exp2-bits scale folded into q
B_EXP2 = (7.0 - C_BIAS * LOG2E - 0.043) * 8.0
ACT_SCALE = SCALE / A_Q

# exp tiles computed on the vector engine (uint8/e4m3 exp2 bit trick)
# instead of scalar ACT, to balance the two engines.
USE_SWI = True  # DoubleRowSwInterleave for projections (False: plain DR)

DEBUG_DUMP = True  # extra DRAM dumps of intermediates for HW debugging

DVE_EXP = {(1, 0), (3, 1), (5, 0), (7, 1)}
DVE_EXP_LATE = {(3, 1), (7, 0)}


def build_program():
    nc = bacc.Bacc(trn_type="TRN2", target_bir_lowering=False, debug=False,
                   num_devices=NCORES)

    x2 = nc.dram_tensor("x2", [BPC, C, N], BF16, kind="ExternalInput").ap()
    x8d = nc.dram_tensor("x8d", [BPC, CTP, 128, 2, N], F8E4,
                         kind="ExternalInput").ap()
    x8vd = nc.dram_tensor("x8vd", [BPC, CTP, 128, NT, 256], F8E4,
                          kind="ExternalInput").ap()
    w8d = {name: nc.dram_tensor(name, [CTP, 128, CT, 256], F8E4,
                                kind="ExternalInput").ap()
           for name in ("w8q", "w8k", "w8o")}
    wv8d = nc.dram_tensor("wv8d", [CTP, 128, 2, C], F8E4,
                          kind="ExternalInput").ap()
    bqk_r = nc.dram_tensor("bqk_r", [128, 2 * CT], F32,
                           kind="ExternalInput").ap()
    bo512 = nc.dram_tensor("bo512", [C], BF16, kind="ExternalInput").ap()
    gamma = nc.dram_tensor("gamma", [1], F32, kind="ExternalInput").ap()
    y2 = nc.dram_tensor("y2", [BPC, C, N], BF16, kind="ExternalOutput").ap()
    dbg = {}
    if DEBUG_DUMP:
        dbg["q"] = nc.dram_tensor("dbg_q", [128, N], BF16,
                                  kind="ExternalOutput").ap()
        dbg["k"] = nc.dram_tensor("dbg_k", [128, N], BF16,
                                  kind="ExternalOutput").ap()
        dbg["v"] = nc.dram_tensor("dbg_v", [128, 2, HEADS, VW], F8E4,
                                  kind="ExternalOutput").ap()
        dbg["e"] = nc.dram_tensor("dbg_e", [128, 2, N], F8E4,
                                  kind="ExternalOutput").ap()
        dbg["on"] = nc.dram_tensor("dbg_on", [128, 2, N], F8E4,
                                   kind="ExternalOutput").ap()

    with tile.TileContext(nc) as tc:
        with (
            tc.tile_pool(name="sb", bufs=1) as sb,
            tc.tile_pool(name="ps", bufs=1, space="PSUM") as ps,
        ):
            st = {"xf": {}, "x8": {}, "x8v": {}, "vext": {}, "on": {},
                  "q": {0: [None] * CT, 1: [None] * CT},
                  "k": {0: [None] * CT, 1: [None] * CT}}

            # ---------------- input DMAs ----------------
            def load_b(b, head):
                """Issue all input DMAs for batch b."""
                qs = [nc.sync, nc.scalar, nc.gpsimd]
                x8t = [sb.tile([128, 2, N], F8E4, tag=f"x8{kcp}", bufs=2,
                               name=f"x8_{b}_{kcp}") for kcp in range(CTP)]
                xv_shape = ([128, NT, 256] if USE_SWI else [128, NT, 2, 128])
                xvt = [sb.tile(xv_shape, F8E4, tag=f"xv{kcp}", bufs=2,
                               name=f"xv_{b}_{kcp}") for kcp in range(CTP)]
                xft = sb.tile([128, CT, N], BF16, tag="xf", bufs=2,
                              name=f"xf{b}")
                if head:
                    # head: q/k proj needs x8 first; halve x8 tiles across
                    # queues for faster first-arrival
                    for kcp in range(CTP):
                        for half in range(2):
                            qs[(2 * kcp + half) % 3].dma_start(
                                out=x8t[kcp][:, half, :],
                                in_=x8d[b, kcp, :, half, :])
                    for kcp in range(CTP):
                        qs[kcp % 3].dma_start(out=xvt[kcp],
                                              in_=x8vd[b, kcp])
                    qs[2].dma_start(out=xft, in_=bass.AP(
                        tensor=x2.tensor, offset=x2.offset + b * C * N,
                        ap=[[N, 128], [128 * N, CT], [1, N]]))
                else:
                    nc.sync.dma_start(out=x8t[0], in_=x8d[b, 0])
                    nc.scalar.dma_start(out=x8t[1], in_=x8d[b, 1])
                    nc.sync.dma_start(out=xvt[0], in_=x8vd[b, 0])
                    nc.scalar.dma_start(out=xvt[1], in_=x8vd[b, 1])
                    nc.gpsimd.dma_start(out=xft, in_=bass.AP(
                        tensor=x2.tensor, offset=x2.offset + b * C * N,
                        ap=[[N, 128], [128 * N, CT], [1, N]]))
                st["x8"][b] = x8t
                st["x8v"][b] = xvt
                st["xf"][b] = xft

            load_b(0, head=True)

            # PE warm-up: tiny matmuls during the head DMA wait release
            # the HAM clock throttle before real work arrives
            wu_sb = sb.tile([128, 128], BF16, tag="wu")
            nc.gpsimd.memset(wu_sb, 0.25)
            wu_rhs = sb.tile([128, 512], BF16, tag="wur")
            nc.gpsimd.memset(wu_rhs, 0.25)
            for wi in range(20):
                wup = ps.tile([128, 512], F32, tag="pq", bufs=2,
                              name=f"warmup{wi}")
                nc.tensor.matmul(wup, lhsT=wu_sb, rhs=wu_rhs,
                                 start=True, stop=True)

            # weights
            w_sb = {}
            _dmae = [nc.scalar, nc.gpsimd, nc.sync]
            for i, name in enumerate(("w8q", "w8k", "w8o")):
                shape = ([128, CTP, CT, 256] if USE_SWI
                         else [128, CTP, CT, 2, 128])
                t = sb.tile(shape, F8E4, tag=name)
                for kcp in range(CTP):
                    _dmae[(i + kcp) % 3].dma_start(
                        out=t[:, kcp], in_=w8d[name][kcp])
                w_sb[name] = t
            wv_sb = sb.tile([128, CTP, 2, C], F8E4, tag="wv8")
            for kcp in range(CTP):
                _dmae[(3 + kcp) % 3].dma_start(out=wv_sb[:, kcp],
                                               in_=wv8d[kcp])

            bqk_sb = sb.tile([128, 2 * CT], F32, tag="bqk")
            nc.sync.dma_start(out=bqk_sb, in_=bqk_r)
            bo_sb = sb.tile([1, C], BF16, tag="bo512")
            nc.sync.dma_start(
                out=bo_sb,
                in_=bass.AP(tensor=bo512.tensor, offset=bo512.offset,
                            ap=[[0, 1]] + list(bo512.ap)))
            gam_sb = sb.tile([128, 1], F32, tag="gam")
            nc.sync.dma_start(
                out=gam_sb,
                in_=bass.AP(tensor=gamma.tensor, offset=gamma.offset,
                            ap=[[0, 128]] + list(gamma.ap)))
            gam512_sb = sb.tile([128, 1], F32, tag="gam512")
            nc.vector.tensor_scalar(gam512_sb, gam_sb, 1.0 / (S_W * S_V),
                                    None, ALU.mult)
            nbias_sb = sb.tile([128, 1], F32, tag="nbias")
            nc.gpsimd.memset(nbias_sb, -C_BIAS)
            ones1 = sb.tile([1, HD], BF16, tag="ones1")
            nc.gpsimd.memset(ones1, 1.0)
            ones512 = sb.tile([1, C], BF16, tag="ones512")
            nc.gpsimd.memset(ones512, 1.0)

            # v tiles: [128 j, 2 ko, 8 h, VW] fp8e4m3 per jt-pair; per head:
            # 64 d values, the ones row (denominator trick) at 64, pad at 65.
            for bb in range(BPC):
                for ntp in range(NTP):
                    t = sb.tile([128, 2, HEADS, VW], F8E4, tag=f"v{ntp}",
                                name=f"vext{bb}_{ntp}", bufs=2)
                    nc.gpsimd.memset(t[:, :, :, HD:HD + 1], 1.0)
                    nc.gpsimd.memset(t[:, :, :, HD + 1:VW], 0.0)
                    st["vext"][(bb, ntp)] = t

            # ---------------- projections ----------------
            def proj_qk_chunk(b, wname, ot, nch):
                dstkey = "q" if wname == "w8q" else "k"
                if nch == 0:
                    st[dstkey][b][ot] = sb.tile(
                        [128, N], BF16, tag=f"{wname}o{ot}", bufs=2,
                        name=f"{dstkey}{b}_{ot}")
                t = st[dstkey][b][ot]
                p = ps.tile([128, 512], F32, tag="pq", bufs=2,
                            name=f"pj{b}{wname}{ot}{nch}")
                for kcp in range(CTP):
                    nc.tensor.matmul(
                        p,
                        lhsT=w_sb[wname][:, kcp, ot],
                        rhs=st["x8"][b][kcp][:, :, nch * 512:(nch + 1) * 512],
                        start=(kcp == 0), stop=(kcp == CTP - 1),
                        perf_mode=DRSWI if USE_SWI else DR,
                    )
                if dstkey == "q":
                    nc.vector.tensor_scalar(
                        t[:, nch * 512:(nch + 1) * 512], p,
                        bqk_sb[:, ot:ot + 1], A_Q / S_W, ALU.add, ALU.mult)
                else:
                    nc.vector.tensor_scalar(
                        t[:, nch * 512:(nch + 1) * 512], p,
                        bqk_sb[:, CT + ot:CT + ot + 1], 1.0 / S_W,
                        ALU.add, ALU.mult)

            def proj_qk_group(b, wname, ot):
                for nch in range(NCH):
                    proj_qk_chunk(b, wname, ot, nch)

            def proj_v_group(b, nt):
                p = ps.tile([128, 512], F32, tag="pq", bufs=2,
                            name=f"pv{b}{nt}")
                for kcp in range(CTP):
                    nc.tensor.matmul(
                        p,
                        lhsT=st["x8v"][b][kcp][:, nt],
                        rhs=wv_sb[:, kcp, :, :],
                        start=(kcp == 0), stop=(kcp == CTP - 1),
                        perf_mode=DRSWI if USE_SWI else DR,
                    )
                nc.scalar.activation(
                    st["vext"][(b, nt // 2)][:, nt % 2, :, 0:HD],
                    p.rearrange("p (h d) -> p h d", h=HEADS),
                    AF.Copy, scale=S_V / S_W)

            def alloc_on(b):
                st["on"][b] = [sb.tile([128, 2, N], F8E4, tag=f"on{kcp}",
                                       name=f"on{b}_{kcp}", bufs=2)
                               for kcp in range(CTP)]

            def outproj_group(b, ot, nch):
                p = ps.tile([128, 512], F32, tag="pq", bufs=2,
                            name=f"po{b}{ot}{nch}")
                for kcp in range(CTP):
                    nc.tensor.matmul(
                        p,
                        lhsT=w_sb["w8o"][:, kcp, ot],
                        rhs=st["on"][b][kcp][:, :, nch * 512:(nch + 1) * 512],
                        start=(kcp == 0), stop=False,
                        perf_mode=DRSWI if USE_SWI else DR,
                        skip_group_check=True,
                    )
                nc.tensor.matmul(
                    p, lhsT=bo_sb[0:1, ot * 128:(ot + 1) * 128],
                    rhs=ones512[0:1, 0:512],
                    start=False, stop=True, skip_group_check=True)
                if nch == 0:
                    st[("y", b, ot)] = sb.tile([128, N], BF16, tag="y",
                                               bufs=2, name=f"y{b}{ot}")
                yt = st[("y", b, ot)]
                nc.vector.scalar_tensor_tensor(
                    yt[:, nch * 512:(nch + 1) * 512], p, gam512_sb[:, 0:1],
                    st["xf"][b][:, ot, nch * 512:(nch + 1) * 512],
                    ALU.mult, ALU.add)
                if nch == NCH - 1:
                    nc.gpsimd.dma_start(out=y2[b, ot * 128:(ot + 1) * 128, :],
                                        in_=yt)

            # ---------- attention building blocks ----------
            EXP = {}  # (b, hp) -> [hh][jtp] fp8 expT tiles [128, 2, N]
            PU = {}   # (b, hp, hh) -> [pu_ic0, pu_ic1]

            def qk_exp(b, hp, jt, dve_set):
                """4 QK matmuls (row-tiled head pair) + exp per hh."""
                q_sb, k_sb = st["q"][b], st["k"][b]
                jtp, ko = divmod(jt, 2)
                pe_pair = [ps.tile([128, N], F32, tag="pe", bufs=2,
                                   name=f"pe{b}_{hp}_{jt}_{hh}")
                           for hh in range(2)]
                for ic in range(NCH):
                    for hh in range(2):
                        nc.tensor.matmul(
                            pe_pair[hh][:, ic * 512:(ic + 1) * 512],
                            lhsT=k_sb[hp][hh * 64:(hh + 1) * 64,
                                          jt * 128:(jt + 1) * 128],
                            rhs=q_sb[hp][hh * 64:(hh + 1) * 64,
                                         ic * 512:(ic + 1) * 512],
                            start=True, stop=True,
                        )
                for hh in range(2):
                    if ko == 0:
                        e = sb.tile([128, 2, N], F8E4, tag="exp", bufs=16,
                                    name=f"e{b}_{hp}_{jtp}_{hh}")
                        EXP[(b, hp)][hh].append(e)
                        if (b, hp, jtp, hh) == (0, 0, 0, 0):
                            st["dbg_e"] = e
                    e = EXP[(b, hp)][hh][jtp]
                    if (jt, hh) in dve_set:
                        nc.vector.tensor_scalar(
                            e.bitcast(U8)[:, ko, :], pe_pair[hh],
                            -B_EXP2, B_EXP2, ALU.max, ALU.add)
                    else:
                        nc.scalar.activation(e[:, ko, :], pe_pair[hh],
                                             AF.Exp, scale=ACT_SCALE,
                                             bias=nbias_sb[:, 0:1])

            def dump_e():
                if DEBUG_DUMP and "dbg_e" in st:
                    nc.sync.dma_start(out=dbg["e"], in_=st["dbg_e"])
                    del st["dbg_e"]

            def av_step(b, hp, hh, jtp):
                """One jt-pair DoubleRow step of the AV chain for one head."""
                h = 2 * hp + hh
                if jtp == 0 and (b, hp, hh) not in PU:
                    PU[(b, hp, hh)] = [
                        ps.tile([128, 512], F32, tag="pu", bufs=2,
                                name=f"pu{b}_{h}_{ic}")
                        for ic in range(NCH)]
                pus = PU[(b, hp, hh)]
                expT = EXP[(b, hp)][hh]
                for ic in range(NCH):
                    nc.tensor.matmul(
                        pus[ic][0:HD + 1, :],
                        lhsT=st["vext"][(b, jtp)][:, :, h, 0:HD + 1],
                        rhs=expT[jtp][:, :, ic * 512:(ic + 1) * 512],
                        start=(jtp == 0), stop=(jtp == NTP - 1),
                        perf_mode=DR,
                        skip_group_check=True,
                    )

            def norm_tail(b, hp, hh):
                h = 2 * hp + hh
                kcp, ko = divmod(h // 2, 2)
                p0 = (h % 2) * 64
                on_t = st["on"][b][kcp]
                for ic in range(NCH):
                    pu = PU[(b, hp, hh)][ic]
                    den = sb.tile([1, 512], BF16, tag="den", bufs=2,
                                  name=f"den{b}_{h}_{ic}")
                    nc.scalar.activation(den, pu[HD:HD + 1, :], AF.Copy)
                    rb = ps.tile([HD, 512], F32, tag="pq", bufs=2,
                                 name=f"rb{b}_{h}_{ic}")
                    nc.tensor.matmul(rb, lhsT=ones1, rhs=den,
                                     start=True, stop=True)
                    r_sb = sb.tile([HD, 512], F32, tag="rsb", bufs=2,
                                   name=f"r{b}_{h}_{ic}")
                    nc.vector.reciprocal_approx_fast(out=r_sb, in_=rb)
                    nc.vector.tensor_tensor(
                        on_t[p0:p0 + HD, ko, ic * 512:(ic + 1) * 512],
                        pu[0:HD, :], r_sb, ALU.mult)
                del PU[(b, hp, hh)]

            # ================= emission schedule =================
            # Filler queue: cheap groups scheduled into exp-paced slack.
            # When empty, emit a short dummy-matmul heater burst.
            fillers = []
            _hn = [0]

            def heater(n=3):
                for _ in range(n):
                    _hn[0] += 1
                    hp_ps = ps.tile([128, 512], F32, tag="pq", bufs=2,
                                    name=f"heat{_hn[0]}")
                    nc.tensor.matmul(hp_ps, lhsT=wu_sb, rhs=wu_rhs,
                                     start=True, stop=True)

            def run_filler(n):
                for _ in range(n):
                    if fillers:
                        fillers.pop(0)()
                    else:
                        heater()

            alloc_on(0)
            alloc_on(1)

            # head: q/k for heads 0,1 of batch 0 only, then attention starts
            proj_qk_group(0, "w8q", 0)
            proj_qk_group(0, "w8k", 0)

            # filler order obeys dependencies:
            #  pair (0,0): remaining b0 projections (q/k ot1 first - needed by
            #              pair (0,1) - then all b0 v tiles)
            fillers += [lambda ot=ot, w=w, nch=nch: proj_qk_chunk(0, w, ot, nch)
                        for ot in (1,) for w in ("w8q", "w8k")
                        for nch in range(NCH)]
            fillers += [lambda: load_b(1, head=False)]
            fillers += [lambda nt=nt: proj_v_group(0, nt) for nt in range(NT)]
            fillers += [lambda ot=ot, w=w, nch=nch: proj_qk_chunk(0, w, ot, nch)
                        for ot in (2, 3) for w in ("w8q", "w8k")
                        for nch in range(NCH)]
            #  pairs (0,1)-(0,3): b1 projections
            fillers += [lambda ot=ot, w=w, nch=nch: proj_qk_chunk(1, w, ot, nch)
                        for ot in range(CT) for w in ("w8q", "w8k")
                        for nch in range(NCH)]
            fillers += [lambda nt=nt: proj_v_group(1, nt) for nt in range(NT)]
            #  pairs (1,1)+: b0 out-projection (ready once AV(0,3) done)
            b0_op = [lambda ot=ot, nch=nch: outproj_group(0, ot, nch)
                     for ot in range(CT) for nch in range(NCH)]

            pairs = [(0, hp) for hp in range(4)] + [(1, hp) for hp in range(4)]
            prev = None
            for pi, (b, hp) in enumerate(pairs):
                EXP[(b, hp)] = [[], []]
                if pi == 5:
                    fillers += b0_op

                # per jt slot: non-blocking tensor work first (AV DoubleRow
                # step of the previous pair, filler group), then this pair's
                # QK whose pe-buffer wait gates the in-order tensor queue
                for jt in range(NT):
                    qk_exp(b, hp, jt,
                           DVE_EXP_LATE if pi >= 5 else DVE_EXP)
                    if pi == 0 and jt == 1:
                        dump_e()
                    if prev is not None and jt % 2 == 1:
                        av_step(prev[0], prev[1], 0, jt // 2)
                    run_filler(1)
                if prev is not None:
                    norm_tail(prev[0], prev[1], 0)
                    for jtp in range(NTP):
                        av_step(prev[0], prev[1], 1, jtp)
                        if jtp == 1:
                            run_filler(1)
                    norm_tail(prev[0], prev[1], 1)
                    del EXP[prev]
                prev = (b, hp)

            if DEBUG_DUMP:
                nc.sync.dma_start(out=dbg["q"], in_=st["q"][0][0])
                nc.sync.dma_start(out=dbg["k"], in_=st["k"][0][0])
                nc.sync.dma_start(out=dbg["v"], in_=st["vext"][(0, 0)])
                nc.sync.dma_start(out=dbg["on"], in_=st["on"][0][0])

            tail_pu = ps.tile([128, N], F32, tag="pe", bufs=2,
                              name="tailpu")
            PU[(1, 3, 1)] = [tail_pu[:, 0:512], tail_pu[:, 512:1024]]
            for jtp in range(NTP):
                av_step(1, 3, 0, jtp)
            norm_tail(1, 3, 0)
            for jtp in range(NTP):
                av_step(1, 3, 1, jtp)
            norm_tail(1, 3, 1)
            run_filler(len(fillers))
            # heater before the final out-projection block so it starts at
            # full clock
            heater(6)
            for ot in range(CT):
                for nch in range(NCH):
                    outproj_group(1, ot, nch)
                    heater(2)
    nc.compile()
    return nc


def prep_shared(inputs):
    """Host-side input prep shared across cores (weights, biases)."""
    f8 = ml_dtypes.float8_e4m3
    out = {}
    for name, key in (("w8q", "wq"), ("w8k", "wk"), ("w8o", "wo")):
        wT = np.ascontiguousarray(inputs[key].astype(np.float64).T) * S_W
        wr = wT.reshape(CTP, 2, 128, CT, 128)  # [kcp, ko, ki, ot, m]
        if USE_SWI:
            lay = wr[:, :, :, :, ::-1].transpose(0, 2, 3, 4, 1)
        else:
            lay = wr.transpose(0, 2, 3, 1, 4)  # [kcp, ki, ot, ko, m]
        out[name] = np.ascontiguousarray(
            lay.reshape(CTP, 128, CT, 256)).astype(f8)
    wvT = np.ascontiguousarray(inputs["wv"].astype(np.float64).T) * S_W
    out["wv8d"] = np.ascontiguousarray(
        wvT.reshape(CTP, 2, 128, C).transpose(0, 2, 1, 3)).astype(f8)
    bq = inputs["bq"].astype(np.float64)
    bk = inputs["bk"].astype(np.float64)
    bqk = np.empty((128, 2 * CT), dtype=np.float32)
    bqk[:, 0:CT] = (S_W * bq).reshape(CT, 128).T
    bqk[:, CT:2 * CT] = (S_W * bk).reshape(CT, 128).T
    out["bqk_r"] = bqk
    bo_eff = inputs["bo"].astype(np.float64) + \
        inputs["wo"].astype(np.float64) @ inputs["bv"].astype(np.float64)
    out["bo512"] = (S_W * S_V * bo_eff).astype(ml_dtypes.bfloat16)
    out["gamma"] = np.ascontiguousarray(inputs["gamma"].astype(np.float32))
    return out


def prep_core(inputs, core):
    """Host-side per-core x prep: bf16 residual + fp8 matmul layouts."""
    f8 = ml_dtypes.float8_e4m3
    x = np.asarray(inputs["x"], dtype=np.float32)
    B = x.shape[0]
    xr = x.reshape(B, C, N)[core * BPC:(core + 1) * BPC]
    out = {"x2": np.ascontiguousarray(xr).astype(ml_dtypes.bfloat16)}
    x4 = xr.reshape(BPC, CT, 128, N).astype(f8)  # [b, ct, ki, n]
    out["x8d"] = np.ascontiguousarray(
        x4.reshape(BPC, CTP, 2, 128, N).transpose(0, 1, 3, 2, 4))
    xv = x4.reshape(BPC, CTP, 2, 128, NT, 128)  # [b, kcp, ko, ki, jt, j]
    if USE_SWI:
        xvl = xv[..., ::-1].transpose(0, 1, 3, 4, 5, 2)
    else:
        xvl = xv.transpose(0, 1, 3, 4, 2, 5)  # [b, kcp, ki, jt, ko, j]
    out["x8vd"] = np.ascontiguousarray(xvl.reshape(BPC, CTP, 128, NT, 256))
    return out


def make_in_maps(inputs):
    shared = prep_shared(inputs)
    return [dict(shared, **prep_core(inputs, core))
            for core in range(NCORES)]


_PROGRAM = None


def _get_program():
    global _PROGRAM
    if _PROGRAM is None:
        _PROGRAM = build_program()
    return _PROGRAM


def kernel(**inputs):
    x = np.asarray(inputs["x"])
    B, c, H, W = x.shape
    assert (c, H * W) == (C, N)
    in_maps = make_in_maps(inputs)
    nc = _get_program()
    res = run_bass_kernel_spmd(nc, in_maps, list(range(NCORES)))
    y = np.concatenate([res.results[i]["y2"].astype(np.float32)
                        for i in range(NCORES)], axis=0)
    return y.reshape(B, C, H, W)


if __name__ == "__main__":
    rng = np.random.default_rng(0)
    ins = {
        "x": rng.standard_normal((16, C, 32, 32), dtype=np.float32),
        "wq": rng.standard_normal((C, C), dtype=np.float32) / 23,
        "bq": rng.standard_normal((C,), dtype=np.float32) / 23,
        "wk": rng.standard_normal((C, C), dtype=np.float32) / 23,
        "bk": rng.standard_normal((C,), dtype=np.float32) / 23,
        "wv": rng.standard_normal((C, C), dtype=np.float32) / 23,
        "bv": rng.standard_normal((C,), dtype=np.float32) / 23,
        "wo": rng.standard_normal((C, C), dtype=np.float32) / 23,
        "bo": rng.standard_normal((C,), dtype=np.float32) / 23,
        "gamma": np.full((1,), 0.1, dtype=np.float32),
    }
    out = kernel(**ins)
    print("kernel ran, out shape", out.shape)


# revision 10
# speedup vs baseline: 1.0001x; 1.0001x over previous
"""Multi-head self-attention 2d kernel for 8 trn2 NeuronCores.

Sharding: data-parallel over batch B=16 -> 2 batches per core.

v2: all four projections run as fp8e4m3 DoubleRowSwInterleave matmuls
(host pre-interleaves the stationary operand, so LDWEIGHTS reads
contiguously and each 512-col matmul covers two 128-deep k-tiles).  exp
tiles are fp8e4m3 (3 mantissa bits vs e5m2's 2), which pays for the fp8
projection noise.  bv is folded into bo on the host (bo_eff = bo+wo@bv),
so the v write is a scaled psum->sbuf copy on the scalar engine.  The
out-projection bias enters PSUM via a K=1 ones matmul and the
scale+residual is one scalar_tensor_tensor on the vector engine.

Per-core dataflow (per batch):
  q = (S_W*(wq8@x8) + S_W*bq)*(A_Q/S_W) -> [C, N] bf16   (A_Q folds the
      e4m3 exp2-bit-trick scale into q)
  k analogously (scale 1/S_W)
  vT tiles [N, 2, 8, 66] fp8e4m3 (64 d + ones row + pad per head),
      written by scalar-engine Copy with scale S_V/S_W
  per head pair hp (software-pipelined one pair behind):
    eT[j, i] = k_h.T @ q_h            (bf16, K=64, row-tiled head pair)
    expT = exp(SCALE*eT - C_BIAS) fp8e4m3  (scalar ACT; some tiles via a
           uint8 exp2 bit trick on the vector engine)
    pu[0:65, i] = v_h.T @ expT        (fp8 DoubleRow over jt pairs;
                                       row 64 = denominator)
    r = 1/denom broadcast via K=1 bf16 matmul + fast reciprocal
    on8 = pu[0:64] * r -> fp8e4m3 (= S_V * out_norm)
  y = (wo8@on8 + 512*bo_eff)*gamma/512 + x   (fp8 DR matmuls, K=1 bias
      matmul, one scalar_tensor_tensor for scale+residual)
"""

import sys

for _p in ("/opt/trn_rl_repo",):
    if _p not in sys.path:
        sys.path.insert(0, _p)

import numpy as np
import ml_dtypes

import concourse.bass as bass
from concourse import bacc
import concourse.mybir as mybir
import concourse.tile as tile
from concourse.bass_utils import run_bass_kernel_spmd

F32 = mybir.dt.float32
BF16 = mybir.dt.bfloat16
F8E4 = mybir.dt.float8e4
U8 = mybir.dt.uint8
AF = mybir.ActivationFunctionType
ALU = mybir.AluOpType
DR = mybir.MatmulPerfMode.DoubleRow
DRSWI = mybir.MatmulPerfMode.DoubleRowSwInterleave

C = 512
N = 1024
HEADS = 8
HD = C // HEADS  # 64
SCALE = HD ** -0.5
CT = C // 128  # 4 channel tiles
CTP = CT // 2  # 2 channel tile pairs (fp8 DR k-tile pairs)
NT = N // 128  # 8 spatial tiles
NTP = NT // 2  # 4 spatial tile pairs
NCH = N // 512  # 2 free-dim chunks
BPC = 2  # batches per core
NCORES = 8
VW = HD + 2  # v tile row stride per head: 64 d + ones + pad
S_W = 16.0  # weight quantization scale (w8 = S_W * w in e4m3)
S_V = 32.0  # v quantization scale (v8 = S_V * v); on8 = S_V * out_norm
C_BIAS = 3.0  # exp bias, cancels in the normalization (keeps exp < 240, e4m3-safe)
LOG2E = float(np.log2(np.e))
A_Q = SCALE * LOG2E * 8.0  # e4m3 exp2-bits scale folded into q
B_EXP2 = (7.0 - C_BIAS * LOG2E - 0.043) * 8.0
ACT_SCALE = SCALE / A_Q

# exp tiles computed on the vector engine (uint8/e4m3 exp2 bit trick)
# instead of scalar ACT, to balance the two engines.
USE_SWI = True  # DoubleRowSwInterleave for projections (False: plain DR)
HEAT_WARMUP = 4   # warm-up matmuls at kernel start
HEAT_SCALE = 0.0  # heater burst multiplier (0 disables)

DEBUG_DUMP = True  # extra DRAM dumps of intermediates for HW debugging

DVE_EXP = {(1, 0), (3, 1), (5, 0), (7, 1)}
DVE_EXP_LATE = {(3, 1), (7, 0)}


def build_program():
    nc = bacc.Bacc(trn_type="TRN2", target_bir_lowering=False, debug=False,
                   num_devices=NCORES)

    x2 = nc.dram_tensor("x2", [BPC, C, N], BF16, kind="ExternalInput").ap()
    x8d = nc.dram_tensor("x8d", [BPC, CTP, 128, 2, N], F8E4,
                         kind="ExternalInput").ap()
    x8vd = nc.dram_tensor("x8vd", [BPC, CTP, 128, NT, 256], F8E4,
                          kind="ExternalInput").ap()
    w8d = {name: nc.dram_tensor(name, [CTP, 128, CT, 256], F8E4,
                                kind="ExternalInput").ap()
           for name in ("w8q", "w8k", "w8o")}
    wv8d = nc.dram_tensor("wv8d", [CTP, 128, 2, C], F8E4,
                          kind="ExternalInput").ap()
    bqk_r = nc.dram_tensor("bqk_r", [128, 2 * CT], F32,
                           kind="ExternalInput").ap()
    bo512 = nc.dram_tensor("bo512", [C], BF16, kind="ExternalInput").ap()
    gamma = nc.dram_tensor("gamma", [1], F32, kind="ExternalInput").ap()
    y2 = nc.dram_tensor("y2", [BPC, C, N], BF16, kind="ExternalOutput").ap()
    dbg = {}
    if DEBUG_DUMP:
        dbg["q"] = nc.dram_tensor("dbg_q", [128, N], BF16,
                                  kind="ExternalOutput").ap()
        dbg["k"] = nc.dram_tensor("dbg_k", [128, N], BF16,
                                  kind="ExternalOutput").ap()
        dbg["v"] = nc.dram_tensor("dbg_v", [128, 2, HEADS, VW], F8E4,
                                  kind="ExternalOutput").ap()
        dbg["e"] = nc.dram_tensor("dbg_e", [128, 2, N], F8E4,
                                  kind="ExternalOutput").ap()
        dbg["on"] = nc.dram_tensor("dbg_on", [128, 2, N], F8E4,
                                   kind="ExternalOutput").ap()

    with tile.TileContext(nc) as tc:
        with (
            tc.tile_pool(name="sb", bufs=1) as sb,
            tc.tile_pool(name="ps", bufs=1, space="PSUM") as ps,
        ):
            st = {"xf": {}, "x8": {}, "x8v": {}, "vext": {}, "on": {},
                  "q": {0: [None] * CT, 1: [None] * CT},
                  "k": {0: [None] * CT, 1: [None] * CT}}

            # ---------------- input DMAs ----------------
            def load_b(b, head):
                """Issue all input DMAs for batch b."""
                qs = [nc.sync, nc.scalar, nc.gpsimd]
                x8t = [sb.tile([128, 2, N], F8E4, tag=f"x8{kcp}", bufs=2,
                               name=f"x8_{b}_{kcp}") for kcp in range(CTP)]
                xv_shape = ([128, NT, 256] if USE_SWI else [128, NT, 2, 128])
                xvt = [sb.tile(xv_shape, F8E4, tag=f"xv{kcp}", bufs=2,
                               name=f"xv_{b}_{kcp}") for kcp in range(CTP)]
                xft = sb.tile([128, CT, N], BF16, tag="xf", bufs=2,
                              name=f"xf{b}")
                if head:
                    # head: q/k proj needs x8 first; halve x8 tiles across
                    # queues for faster first-arrival
                    for kcp in range(CTP):
                        for half in range(2):
                            qs[(2 * kcp + half) % 3].dma_start(
                                out=x8t[kcp][:, half, :],
                                in_=x8d[b, kcp, :, half, :])
                    for kcp in range(CTP):
                        qs[kcp % 3].dma_start(out=xvt[kcp],
                                              in_=x8vd[b, kcp])
                    qs[2].dma_start(out=xft, in_=bass.AP(
                        tensor=x2.tensor, offset=x2.offset + b * C * N,
                        ap=[[N, 128], [128 * N, CT], [1, N]]))
                else:
                    nc.sync.dma_start(out=x8t[0], in_=x8d[b, 0])
                    nc.scalar.dma_start(out=x8t[1], in_=x8d[b, 1])
                    nc.sync.dma_start(out=xvt[0], in_=x8vd[b, 0])
                    nc.scalar.dma_start(out=xvt[1], in_=x8vd[b, 1])
                    nc.gpsimd.dma_start(out=xft, in_=bass.AP(
                        tensor=x2.tensor, offset=x2.offset + b * C * N,
                        ap=[[N, 128], [128 * N, CT], [1, N]]))
                st["x8"][b] = x8t
                st["x8v"][b] = xvt
                st["xf"][b] = xft

            load_b(0, head=True)

            # PE warm-up: tiny matmuls during the head DMA wait release
            # the HAM clock throttle before real work arrives
            wu_sb = sb.tile([128, 128], BF16, tag="wu")
            nc.gpsimd.memset(wu_sb, 0.25)
            wu_rhs = sb.tile([128, 512], BF16, tag="wur")
            nc.gpsimd.memset(wu_rhs, 0.25)
            for wi in range(HEAT_WARMUP):
                wup = ps.tile([128, 512], F32, tag="pq", bufs=2,
                              name=f"warmup{wi}")
                nc.tensor.matmul(wup, lhsT=wu_sb, rhs=wu_rhs,
                                 start=True, stop=True)

            # weights
            w_sb = {}
            _dmae = [nc.scalar, nc.gpsimd, nc.sync]
            for i, name in enumerate(("w8q", "w8k", "w8o")):
                shape = ([128, CTP, CT, 256] if USE_SWI
                         else [128, CTP, CT, 2, 128])
                t = sb.tile(shape, F8E4, tag=name)
                for kcp in range(CTP):
                    _dmae[(i + kcp) % 3].dma_start(
                        out=t[:, kcp], in_=w8d[name][kcp])
                w_sb[name] = t
            wv_sb = sb.tile([128, CTP, 2, C], F8E4, tag="wv8")
            for kcp in range(CTP):
                _dmae[(3 + kcp) % 3].dma_start(out=wv_sb[:, kcp],
                                               in_=wv8d[kcp])

            bqk_sb = sb.tile([128, 2 * CT], F32, tag="bqk")
            nc.sync.dma_start(out=bqk_sb, in_=bqk_r)
            bo_sb = sb.tile([1, C], BF16, tag="bo512")
            nc.sync.dma_start(
                out=bo_sb,
                in_=bass.AP(tensor=bo512.tensor, offset=bo512.offset,
                            ap=[[0, 1]] + list(bo512.ap)))
            gam_sb = sb.tile([128, 1], F32, tag="gam")
            nc.sync.dma_start(
                out=gam_sb,
                in_=bass.AP(tensor=gamma.tensor, offset=gamma.offset,
                            ap=[[0, 128]] + list(gamma.ap)))
            gam512_sb = sb.tile([128, 1], F32, tag="gam512")
            nc.vector.tensor_scalar(gam512_sb, gam_sb, 1.0 / (S_W * S_V),
                                    None, ALU.mult)
            nbias_sb = sb.tile([128, 1], F32, tag="nbias")
            nc.gpsimd.memset(nbias_sb, -C_BIAS)
            ones1 = sb.tile([1, HD], BF16, tag="ones1")
            nc.gpsimd.memset(ones1, 1.0)
            ones512 = sb.tile([1, C], BF16, tag="ones512")
            nc.gpsimd.memset(ones512, 1.0)

            # v tiles: [128 j, 2 ko, 8 h, VW] fp8e4m3 per jt-pair; per head:
            # 64 d values, the ones row (denominator trick) at 64, pad at 65.
            for bb in range(BPC):
                for ntp in range(NTP):
                    t = sb.tile([128, 2, HEADS, VW], F8E4, tag=f"v{ntp}",
                                name=f"vext{bb}_{ntp}", bufs=2)
                    nc.gpsimd.memset(t[:, :, :, HD:HD + 1], 1.0)
                    nc.gpsimd.memset(t[:, :, :, HD + 1:VW], 0.0)
                    st["vext"][(bb, ntp)] = t

            # ---------------- projections ----------------
            def proj_qk_chunk(b, wname, ot, nch):
                dstkey = "q" if wname == "w8q" else "k"
                if nch == 0:
                    st[dstkey][b][ot] = sb.tile(
                        [128, N], BF16, tag=f"{wname}o{ot}", bufs=2,
                        name=f"{dstkey}{b}_{ot}")
                t = st[dstkey][b][ot]
                p = ps.tile([128, 512], F32, tag="pq", bufs=2,
                            name=f"pj{b}{wname}{ot}{nch}")
                for kcp in range(CTP):
                    nc.tensor.matmul(
                        p,
                        lhsT=w_sb[wname][:, kcp, ot],
                        rhs=st["x8"][b][kcp][:, :, nch * 512:(nch + 1) * 512],
                        start=(kcp == 0), stop=(kcp == CTP - 1),
                        perf_mode=DRSWI if USE_SWI else DR,
                    )
                if dstkey == "q":
                    nc.vector.tensor_scalar(
                        t[:, nch * 512:(nch + 1) * 512], p,
                        bqk_sb[:, ot:ot + 1], A_Q / S_W, ALU.add, ALU.mult)
                else:
                    nc.vector.tensor_scalar(
                        t[:, nch * 512:(nch + 1) * 512], p,
                        bqk_sb[:, CT + ot:CT + ot + 1], 1.0 / S_W,
                        ALU.add, ALU.mult)

            def proj_qk_group(b, wname, ot):
                for nch in range(NCH):
                    proj_qk_chunk(b, wname, ot, nch)

            def proj_v_group(b, nt):
                p = ps.tile([128, 512], F32, tag="pq", bufs=2,
                            name=f"pv{b}{nt}")
                for kcp in range(CTP):
                    nc.tensor.matmul(
                        p,
                        lhsT=st["x8v"][b][kcp][:, nt],
                        rhs=wv_sb[:, kcp, :, :],
                        start=(kcp == 0), stop=(kcp == CTP - 1),
                        perf_mode=DRSWI if USE_SWI else DR,
                    )
                nc.scalar.activation(
                    st["vext"][(b, nt // 2)][:, nt % 2, :, 0:HD],
                    p.rearrange("p (h d) -> p h d", h=HEADS),
                    AF.Copy, scale=S_V / S_W)

            def alloc_on(b):
                st["on"][b] = [sb.tile([128, 2, N], F8E4, tag=f"on{kcp}",
                                       name=f"on{b}_{kcp}", bufs=2)
                               for kcp in range(CTP)]

            def outproj_group(b, ot, nch):
                p = ps.tile([128, 512], F32, tag="pq", bufs=2,
                            name=f"po{b}{ot}{nch}")
                for kcp in range(CTP):
                    nc.tensor.matmul(
                        p,
                        lhsT=w_sb["w8o"][:, kcp, ot],
                        rhs=st["on"][b][kcp][:, :, nch * 512:(nch + 1) * 512],
                        start=(kcp == 0), stop=False,
                        perf_mode=DRSWI if USE_SWI else DR,
                        skip_group_check=True,
                    )
                nc.tensor.matmul(
                    p, lhsT=bo_sb[0:1, ot * 128:(ot + 1) * 128],
                    rhs=ones512[0:1, 0:512],
                    start=False, stop=True, skip_group_check=True)
                if nch == 0:
                    st[("y", b, ot)] = sb.tile([128, N], BF16, tag="y",
                                               bufs=2, name=f"y{b}{ot}")
                yt = st[("y", b, ot)]
                nc.vector.scalar_tensor_tensor(
                    yt[:, nch * 512:(nch + 1) * 512], p, gam512_sb[:, 0:1],
                    st["xf"][b][:, ot, nch * 512:(nch + 1) * 512],
                    ALU.mult, ALU.add)
                if nch == NCH - 1:
                    nc.gpsimd.dma_start(out=y2[b, ot * 128:(ot + 1) * 128, :],
                                        in_=yt)

            # ---------- attention building blocks ----------
            EXP = {}  # (b, hp) -> [hh][jtp] fp8 expT tiles [128, 2, N]
            PU = {}   # (b, hp, hh) -> [pu_ic0, pu_ic1]

            def qk_exp(b, hp, jt, dve_set):
                """4 QK matmuls (row-tiled head pair) + exp per hh."""
                q_sb, k_sb = st["q"][b], st["k"][b]
                jtp, ko = divmod(jt, 2)
                pe_pair = [ps.tile([128, N], F32, tag="pe", bufs=2,
                                   name=f"pe{b}_{hp}_{jt}_{hh}")
                           for hh in range(2)]
                for ic in range(NCH):
                    for hh in range(2):
                        nc.tensor.matmul(
                            pe_pair[hh][:, ic * 512:(ic + 1) * 512],
                            lhsT=k_sb[hp][hh * 64:(hh + 1) * 64,
                                          jt * 128:(jt + 1) * 128],
                            rhs=q_sb[hp][hh * 64:(hh + 1) * 64,
                                         ic * 512:(ic + 1) * 512],
                            start=True, stop=True,
                        )
                for hh in range(2):
                    if ko == 0:
                        e = sb.tile([128, 2, N], F8E4, tag="exp", bufs=16,
                                    name=f"e{b}_{hp}_{jtp}_{hh}")
                        EXP[(b, hp)][hh].append(e)
                        if (b, hp, jtp, hh) == (0, 0, 0, 0):
                            st["dbg_e"] = e
                    e = EXP[(b, hp)][hh][jtp]
                    if (jt, hh) in dve_set:
                        nc.vector.tensor_scalar(
                            e.bitcast(U8)[:, ko, :], pe_pair[hh],
                            -B_EXP2, B_EXP2, ALU.max, ALU.add)
                    else:
                        nc.scalar.activation(e[:, ko, :], pe_pair[hh],
                                             AF.Exp, scale=ACT_SCALE,
                                             bias=nbias_sb[:, 0:1])

            def dump_e():
                if DEBUG_DUMP and "dbg_e" in st:
                    nc.sync.dma_start(out=dbg["e"], in_=st["dbg_e"])
                    del st["dbg_e"]

            def av_step(b, hp, hh, jtp):
                """One jt-pair DoubleRow step of the AV chain for one head."""
                h = 2 * hp + hh
                if jtp == 0 and (b, hp, hh) not in PU:
                    PU[(b, hp, hh)] = [
                        ps.tile([128, 512], F32, tag="pu", bufs=2,
                                name=f"pu{b}_{h}_{ic}")
                        for ic in range(NCH)]
                pus = PU[(b, hp, hh)]
                expT = EXP[(b, hp)][hh]
                for ic in range(NCH):
                    nc.tensor.matmul(
                        pus[ic][0:HD + 1, :],
                        lhsT=st["vext"][(b, jtp)][:, :, h, 0:HD + 1],
                        rhs=expT[jtp][:, :, ic * 512:(ic + 1) * 512],
                        start=(jtp == 0), stop=(jtp == NTP - 1),
                        perf_mode=DR,
                        skip_group_check=True,
                    )

            def norm_tail(b, hp, hh):
                h = 2 * hp + hh
                kcp, ko = divmod(h // 2, 2)
                p0 = (h % 2) * 64
                on_t = st["on"][b][kcp]
                for ic in range(NCH):
                    pu = PU[(b, hp, hh)][ic]
                    den = sb.tile([1, 512], BF16, tag="den", bufs=2,
                                  name=f"den{b}_{h}_{ic}")
                    nc.scalar.activation(den, pu[HD:HD + 1, :], AF.Copy)
                    rb = ps.tile([HD, 512], F32, tag="pq", bufs=2,
                                 name=f"rb{b}_{h}_{ic}")
                    nc.tensor.matmul(rb, lhsT=ones1, rhs=den,
                                     start=True, stop=True)
                    r_sb = sb.tile([HD, 512], F32, tag="rsb", bufs=2,
                                   name=f"r{b}_{h}_{ic}")
                    nc.vector.reciprocal_approx_fast(out=r_sb, in_=rb)
                    nc.vector.tensor_tensor(
                        on_t[p0:p0 + HD, ko, ic * 512:(ic + 1) * 512],
                        pu[0:HD, :], r_sb, ALU.mult)
                del PU[(b, hp, hh)]

            # ================= emission schedule =================
            # Filler queue: cheap groups scheduled into exp-paced slack.
            # When empty, emit a short dummy-matmul heater burst.
            fillers = []
            _hn = [0]

            def heater(n=3):
                n = int(n * HEAT_SCALE)
                for _ in range(n):
                    _hn[0] += 1
                    hp_ps = ps.tile([128, 512], F32, tag="pq", bufs=2,
                                    name=f"heat{_hn[0]}")
                    nc.tensor.matmul(hp_ps, lhsT=wu_sb, rhs=wu_rhs,
                                     start=True, stop=True)

            def run_filler(n):
                for _ in range(n):
                    if fillers:
                        fillers.pop(0)()
                    else:
                        heater()

            alloc_on(0)
            alloc_on(1)

            # head: q/k for heads 0,1 of batch 0 only, then attention starts
            proj_qk_group(0, "w8q", 0)
            proj_qk_group(0, "w8k", 0)

            # filler order obeys dependencies:
            #  pair (0,0): remaining b0 projections (q/k ot1 first - needed by
            #              pair (0,1) - then all b0 v tiles)
            fillers += [lambda ot=ot, w=w, nch=nch: proj_qk_chunk(0, w, ot, nch)
                        for ot in (1,) for w in ("w8q", "w8k")
                        for nch in range(NCH)]
            fillers += [lambda: load_b(1, head=False)]
            fillers += [lambda nt=nt: proj_v_group(0, nt) for nt in range(NT)]
            fillers += [lambda ot=ot, w=w, nch=nch: proj_qk_chunk(0, w, ot, nch)
                        for ot in (2, 3) for w in ("w8q", "w8k")
                        for nch in range(NCH)]
            #  pairs (0,1)-(0,3): b1 projections
            fillers += [lambda ot=ot, w=w, nch=nch: proj_qk_chunk(1, w, ot, nch)
                        for ot in range(CT) for w in ("w8q", "w8k")
                        for nch in range(NCH)]
            fillers += [lambda nt=nt: proj_v_group(1, nt) for nt in range(NT)]
            #  pairs (1,1)+: b0 out-projection (ready once AV(0,3) done)
            b0_op = [lambda ot=ot, nch=nch: outproj_group(0, ot, nch)
                     for ot in range(CT) for nch in range(NCH)]

            pairs = [(0, hp) for hp in range(4)] + [(1, hp) for hp in range(4)]
            prev = None
            for pi, (b, hp) in enumerate(pairs):
                EXP[(b, hp)] = [[], []]
                if pi == 5:
                    fillers += b0_op

                # per jt slot: non-blocking tensor work first (AV DoubleRow
                # step of the previous pair, filler group), then this pair's
                # QK whose pe-buffer wait gates the in-order tensor queue
                for jt in range(NT):
                    qk_exp(b, hp, jt,
                           DVE_EXP_LATE if pi >= 5 else DVE_EXP)
                    if pi == 0 and jt == 1:
                        dump_e()
                    if prev is not None and jt % 2 == 1:
                        av_step(prev[0], prev[1], 0, jt // 2)
                    run_filler(1)
                if prev is not None:
                    norm_tail(prev[0], prev[1], 0)
                    for jtp in range(NTP):
                        av_step(prev[0], prev[1], 1, jtp)
                        if jtp == 1:
                            run_filler(1)
                    norm_tail(prev[0], prev[1], 1)
                    del EXP[prev]
                prev = (b, hp)

            if DEBUG_DUMP:
                nc.sync.dma_start(out=dbg["q"], in_=st["q"][0][0])
                nc.sync.dma_start(out=dbg["k"], in_=st["k"][0][0])
                nc.sync.dma_start(out=dbg["v"], in_=st["vext"][(0, 0)])
                nc.sync.dma_start(out=dbg["on"], in_=st["on"][0][0])

            tail_pu = ps.tile([128, N], F32, tag="pe", bufs=2,
                              name="tailpu")
            PU[(1, 3, 1)] = [tail_pu[:, 0:512], tail_pu[:, 512:1024]]
            for jtp in range(NTP):
                av_step(1, 3, 0, jtp)
            norm_tail(1, 3, 0)
            for jtp in range(NTP):
                av_step(1, 3, 1, jtp)
            norm_tail(1, 3, 1)
            run_filler(len(fillers))
            # heater before the final out-projection block so it starts at
            # full clock
            heater(6)
            for ot in range(CT):
                for nch in range(NCH):
                    outproj_group(1, ot, nch)
                    heater(2)
    nc.compile()
    return nc


def prep_shared(inputs):
    """Host-side input prep shared across cores (weights, biases)."""
    f8 = ml_dtypes.float8_e4m3
    out = {}
    for name, key in (("w8q", "wq"), ("w8k", "wk"), ("w8o", "wo")):
        wT = np.ascontiguousarray(inputs[key].astype(np.float64).T) * S_W
        wr = wT.reshape(CTP, 2, 128, CT, 128)  # [kcp, ko, ki, ot, m]
        if USE_SWI:
            lay = wr[:, :, :, :, ::-1].transpose(0, 2, 3, 4, 1)
        else:
            lay = wr.transpose(0, 2, 3, 1, 4)  # [kcp, ki, ot, ko, m]
        out[name] = np.ascontiguousarray(
            lay.reshape(CTP, 128, CT, 256)).astype(f8)
    wvT = np.ascontiguousarray(inputs["wv"].astype(np.float64).T) * S_W
    out["wv8d"] = np.ascontiguousarray(
        wvT.reshape(CTP, 2, 128, C).transpose(0, 2, 1, 3)).astype(f8)
    bq = inputs["bq"].astype(np.float64)
    bk = inputs["bk"].astype(np.float64)
    bqk = np.empty((128, 2 * CT), dtype=np.float32)
    bqk[:, 0:CT] = (S_W * bq).reshape(CT, 128).T
    bqk[:, CT:2 * CT] = (S_W * bk).reshape(CT, 128).T
    out["bqk_r"] = bqk
    bo_eff = inputs["bo"].astype(np.float64) + \
        inputs["wo"].astype(np.float64) @ inputs["bv"].astype(np.float64)
    out["bo512"] = (S_W * S_V * bo_eff).astype(ml_dtypes.bfloat16)
    out["gamma"] = np.ascontiguousarray(inputs["gamma"].astype(np.float32))
    return out


def prep_core(inputs, core):
    """Host-side per-core x prep: bf16 residual + fp8 matmul layouts."""
    f8 = ml_dtypes.float8_e4m3
    x = np.asarray(inputs["x"], dtype=np.float32)
    B = x.shape[0]
    xr = x.reshape(B, C, N)[core * BPC:(core + 1) * BPC]
    out = {"x2": np.ascontiguousarray(xr).astype(ml_dtypes.bfloat16)}
    x4 = xr.reshape(BPC, CT, 128, N).astype(f8)  # [b, ct, ki, n]
    out["x8d"] = np.ascontiguousarray(
        x4.reshape(BPC, CTP, 2, 128, N).transpose(0, 1, 3, 2, 4))
    xv = x4.reshape(BPC, CTP, 2, 128, NT, 128)  # [b, kcp, ko, ki, jt, j]
    if USE_SWI:
        xvl = xv[..., ::-1].transpose(0, 1, 3, 4, 5, 2)
    else:
        xvl = xv.transpose(0, 1, 3, 4, 2, 5)  # [b, kcp, ki, jt, ko, j]
    out["x8vd"] = np.ascontiguousarray(xvl.reshape(BPC, CTP, 128, NT, 256))
    return out


def make_in_maps(inputs):
    shared = prep_shared(inputs)
    return [dict(shared, **prep_core(inputs, core))
            for core in range(NCORES)]


_PROGRAM = None


def _get_program():
    global _PROGRAM
    if _PROGRAM is None:
        _PROGRAM = build_program()
    return _PROGRAM


def kernel(**inputs):
    x = np.asarray(inputs["x"])
    B, c, H, W = x.shape
    assert (c, H * W) == (C, N)
    in_maps = make_in_maps(inputs)
    nc = _get_program()
    res = run_bass_kernel_spmd(nc, in_maps, list(range(NCORES)))
    y = np.concatenate([res.results[i]["y2"].astype(np.float32)
                        for i in range(NCORES)], axis=0)
    return y.reshape(B, C, H, W)


if __name__ == "__main__":
    rng = np.random.default_rng(0)
    ins = {
        "x": rng.standard_normal((16, C, 32, 32), dtype=np.float32),
        "wq": rng.standard_normal((C, C), dtype=np.float32) / 23,
        "bq": rng.standard_normal((C,), dtype=np.float32) / 23,
        "wk": rng.standard_normal((C, C), dtype=np.float32) / 23,
        "bk": rng.standard_normal((C,), dtype=np.float32) / 23,
        "wv": rng.standard_normal((C, C), dtype=np.float32) / 23,
        "bv": rng.standard_normal((C,), dtype=np.float32) / 23,
        "wo": rng.standard_normal((C, C), dtype=np.float32) / 23,
        "bo": rng.standard_normal((C,), dtype=np.float32) / 23,
        "gamma": np.full((1,), 0.1, dtype=np.float32),
    }
    out = kernel(**ins)
    print("kernel ran, out shape", out.shape)
